# revision 86
# baseline (speedup 1.0000x reference)
"""BaiChuan attention block (QKV proj + RoPE + causal attention + o_proj) on 8 NeuronCores.

Sharding: tensor-parallel over heads. Each core owns 4 of the 32 heads:
W_pack columns (q/k/v slices) are column-sharded, w_o is row-sharded, and the
8 partial o_proj outputs are summed on the host (cheap f32 reduce) instead of
an on-device all-reduce.

Everything on-device runs in bf16 (fp32 PSUM accumulation). Activations are
kept feature-major ("transposed", [feature, batch*seq]) end to end so that
softmax sums run along the PSUM partition axis and no probability-tile
transposes are needed:
  scoresT[k, q] = K_chunk @ Q_group    (lhsT = KT chunk, rhs = QT group)
  probsT = exp(scoresT * scale)        (softmax scale folded into the ACT
                                        scale operand; no max subtraction:
                                        |scores| <= ~12 for this distribution)
  causal mask  = sliding slice of a constant 0/1 tril tile, applied only to
                 the diagonal chunks, which are also column-trimmed: chunk j
                 of group g only computes q columns >= (j-4g)*128
  outT[d, q]  += V_kd chunk @ probsT   (PSUM accumulate over k chunks)
  acc[k, q]   += probsT                (DVE running sum of prob chunks)
  denom+bcast  = ones[128x128] @ acc   (ONE matmul reduces acc over k AND
                                        broadcasts the denominator to all 128
                                        partitions - replaces per-chunk
                                        ones-row matmuls + a bcast matmul)
  normalize    = reciprocal_approx_fast + multiply on DVE

Engine assignment: TensorE matmuls; ScalarE = Exp + PSUM->SBUF copies (exp and
copy live in the same ACT table set, no reload); DVE = rope, masks, prob
accumulation, normalize; sync HWDGE = phase-1 loads (w1 prioritized over ht);
scalar HWDGE = stores + attention loads; gpsimd SWDGE = constants + rope-swap
reads.

Scheduling for the HAM clock gate (engine streams are static and in-order, so
overlap must be baked into emission order):
  - w_pack is host-packed to match SBUF layout (contiguous 8KB rows, no 2x
    small-descriptor DMA penalty), m=0 chunk shipped first so the first
    matmul starts ~4us in.
  - head 0 (and raw head-1 q/k) preload + rope DURING phase 1, into pools
    allocated before the phase-1 pools so addresses never conflict; the
    first scores matmul issues immediately after the last qkv matmul.
  - attention interleaves pairs of (head, group) j-streams ("riffle") plus a
    2-deep scores pipeline, so PE always has >=4 independent matmuls between
    a scores matmul and the PV matmul that needs its exp'd probs.
  - o_proj m-chunks and denominator matmuls are queued side work, popped
    after each PV step (denominators with priority: PSUM/acc rings recycle
    only after a group's normalize).
"""

import os
from collections import deque
import numpy as np
import ml_dtypes

import concourse.bass as bass
import concourse.tile as tile
import concourse.mybir as mybir
from concourse import bacc
from concourse.bass_utils import run_bass_kernel_spmd

F32 = mybir.dt.float32
BF16 = mybir.dt.bfloat16
AF = mybir.ActivationFunctionType
BF = ml_dtypes.bfloat16

B, S, H = 2, 2048, 4096
BS = B * S                      # 4096 tokens
D = 128                         # head dim
NCORES = 8
NH_LOC = 4                      # heads per core (32 / 8)
HK = H // 128                   # 32 contraction chunks for qkv proj
M_QKV = 3 * NH_LOC              # 12 qkv output row-chunks per core
ST = 512                        # seq tile
NT = BS // ST                   # 8 seq tiles
GP = S // ST                    # 4 q-groups per sequence
ROPE_THETA = 10000.0
SCALE = D ** -0.5

# Pairs of (head, group) whose j-streams are riffled together. b=0 opens with
# a head-0-only runway (head 0 is roped during phase 1) while heads 1..3
# load+rope behind it, chained through a single xload buffer ring.
B0_PAIRS = [[(0, 1), (0, 2)], [(0, 0), (0, 3)], [(1, 0), (1, 1)],
            [(2, 0), (1, 2)], [(3, 0), (2, 1)], [(1, 3), (2, 2)],
            [(3, 1), (2, 3)], [(3, 2)], [(3, 3)]]
B1_PAIRS = [[(0, 0), (1, 0)], [(2, 0), (3, 0)], [(0, 1), (1, 1)],
            [(2, 1), (3, 1)], [(0, 2), (1, 2)], [(2, 2), (3, 2)],
            [(0, 3), (1, 3)], [(2, 3), (3, 3)]]

LAST_RESULT = None              # BassKernelResults of the most recent run (for test.py)


def _riffle(pairs):
    """[(h,g)...] pair groups -> flat (h, g, j, nj) step list, interleaved."""
    steps = []
    for grp_list in pairs:
        streams = [[(h, g, j, 4 * g + 4) for j in range(4 * g + 4)]
                   for h, g in grp_list]
        k = 0
        while any(streams):
            st = streams[k % len(streams)]
            if st:
                steps.append(st.pop(0))
            k += 1
    return steps


def _build_program():
    nc = bacc.Bacc()

    hT = nc.dram_tensor("hT", [H, BS], BF16, kind="ExternalInput")
    w1 = nc.dram_tensor("w1", [128, M_QKV, HK, 128], BF16, kind="ExternalInput")
    wo = nc.dram_tensor("wo", [NH_LOC * 128, H], BF16, kind="ExternalInput")
    cs = nc.dram_tensor("cs", [128, S], BF16, kind="ExternalInput")
    sn = nc.dram_tensor("sn", [128, S], BF16, kind="ExternalInput")
    maskd = nc.dram_tensor("mask", [128, ST], BF16, kind="ExternalInput")
    out = nc.dram_tensor("out", [H, BS], BF16, kind="ExternalOutput")

    with tile.TileContext(nc) as tc:
        with (
            tc.tile_pool(name="cons", bufs=1) as cons,
            tc.tile_pool(name="dram", bufs=1, space="DRAM") as dram,
            tc.tile_pool(name="ps_acc", bufs=5, space="PSUM") as ps_acc,
            tc.tile_pool(name="ps_sc", bufs=3, space="PSUM") as ps_sc_p,
            tc.tile_pool(name="xload", bufs=2) as xload,
            tc.tile_pool(name="eheads", bufs=1) as eheads,
        ):
            # per-(row-chunk, batch) bounce tiles: a head's read then depends
            # only on the four writes that filled its own tile, not on the
            # whole phase-1 write stream (interval tracking coarsens columns).
            qkv_t = [[dram.tile([128, S], BF16, name=f"qkv_{m}_{bb}",
                                tag=f"qkv_{m}_{bb}")
                      for bb in range(B)] for m in range(M_QKV)]
            hT3 = hT.rearrange("(ko p) s -> p ko s", p=128)

            def emit_qk_load(b, h, pool):
                """Load raw q/k straight into the rope-target tiles (rope
                then runs in place); all loads on SP so the ScalarE stream
                stays pure exp+copy."""
                nm = f"{b}_{h}"
                qt = pool.tile([128, S], BF16, tag="qt", name=f"qt_{nm}")
                nc.sync.dma_start(qt[:], qkv_t[h][b][:])
                kt = pool.tile([128, S], BF16, tag="kt", name=f"kt_{nm}")
                nc.sync.dma_start(kt[:], qkv_t[NH_LOC + h][b][:])
                return qt, kt

            def emit_swaps(b, h, eng=None):
                """Partition-swapped q/k copies for the rope sin term."""
                eng = eng or nc.sync
                xqs = xload.tile([128, S], BF16, tag="xqs", name=f"xqs_{b}_{h}")
                eng.dma_start(xqs[0:64, :], qkv_t[h][b][64:128, :])
                eng.dma_start(xqs[64:128, :], qkv_t[h][b][0:64, :])
                xks = xload.tile([128, S], BF16, tag="xks", name=f"xks_{b}_{h}")
                eng.dma_start(xks[0:64, :], qkv_t[NH_LOC + h][b][64:128, :])
                eng.dma_start(xks[64:128, :], qkv_t[NH_LOC + h][b][0:64, :])
                return xqs, xks

            def emit_vkd(b, h, pool, tag="vkd"):
                # DmaTranspose holds its issuing sequencer for the whole
                # transfer (~5us): keep these on SP, which has no compute.
                v_kd = pool.tile([128, S // 128, 128], BF16, tag=tag, name=f"vkd_{b}_{h}")
                nc.sync.dma_start_transpose(v_kd[:], qkv_t[2 * NH_LOC + h][b][:])
                return v_kd

            def rope_ops(qt, kt, xqs, xks):
                """Six one-op in-place rope emitters, interleavable with
                attention's mask/acc DVE traffic."""
                return [
                    lambda: nc.vector.tensor_mul(qt[:], qt[:], cs_sb[:]),
                    lambda: nc.vector.tensor_mul(xqs[:], xqs[:], sn_sb[:]),
                    lambda: nc.vector.tensor_add(qt[:], qt[:], xqs[:]),
                    lambda: nc.vector.tensor_mul(kt[:], kt[:], cs_sb[:]),
                    lambda: nc.vector.tensor_mul(xks[:], xks[:], sn_sb[:]),
                    lambda: nc.vector.tensor_add(kt[:], kt[:], xks[:]),
                ]

            # ---------------- Phase 1: qkvT = w1.T @ hT ----------------
            with (
                tc.tile_pool(name="w1p", bufs=1) as w1p,
                tc.tile_pool(name="htp", bufs=2) as htp,
                tc.tile_pool(name="p1o", bufs=2) as p1o,
            ):
                cs_sb = cons.tile([128, S], BF16, tag="cs")
                nc.scalar.dma_start(cs_sb[:], cs[:])
                sn_sb = cons.tile([128, S], BF16, tag="sn")
                nc.scalar.dma_start(sn_sb[:], sn[:])
                mask_sb = cons.tile([128, ST], BF16, tag="mask")
                nc.scalar.dma_start(mask_sb[:], maskd[:])
                ones128 = cons.tile([128, 128], BF16, tag="ones128")
                nc.vector.memset(ones128[:], 1.0)

                w_sb = w1p.tile([128, M_QKV, HK, 128], BF16, tag="w1")

                def emit_ht(t):
                    ht = htp.tile([128, HK, ST], BF16, tag="ht", name=f"ht_{t}")
                    for oct_ in range(4):
                        nc.sync.dma_start(
                            ht[:, oct_ * 8:(oct_ + 1) * 8],
                            hT3[:, oct_ * 8:(oct_ + 1) * 8, t * ST:(t + 1) * ST])
                    return ht

                # first w chunk in ko-quarters riffled with ht0's octs so the
                # very first matmuls are gated by ~1MB, not ~5MB
                nc.sync.dma_start(w_sb[:, 0, 0:8], w1[:, 0, 0:8])
                ht0 = htp.tile([128, HK, ST], BF16, tag="ht", name="ht_0")
                for oct_ in range(4):
                    nc.sync.dma_start(ht0[:, oct_ * 8:(oct_ + 1) * 8],
                                      hT3[:, oct_ * 8:(oct_ + 1) * 8, 0:ST])
                    if oct_ < 3:
                        nc.sync.dma_start(w_sb[:, 0, (oct_ + 1) * 8:(oct_ + 2) * 8],
                                          w1[:, 0, (oct_ + 1) * 8:(oct_ + 2) * 8])
                for m in range(1, M_QKV):
                    nc.sync.dma_start(w_sb[:, m], w1[:, m])

                tiles0 = {}
                ht_next = ht0
                for t in range(NT):
                    ht = ht_next
                    if t < NT - 1:
                        # prefetch emission: ht(t+1) enters the sync FIFO
                        # ahead of tile t's qkv stores
                        ht_next = emit_ht(t + 1)
                    for m in range(M_QKV):
                        ps = ps_acc.tile([128, ST], F32, tag="acc",
                                         name=f"ps_q_{t}_{m}")
                        for ko in range(HK):
                            nc.tensor.matmul(
                                ps[:], w_sb[:, m, ko], ht[:, ko],
                                start=(ko == 0), stop=(ko == HK - 1))
                        ob = p1o.tile([128, ST], BF16, tag="ob")
                        nc.scalar.activation(ob[:], ps[:], AF.Copy)
                        tc_, tb = t % GP, t // GP
                        # t=7 stores issue from the ACT queue (woven between
                        # its own copies, done pre-transition) so the b0 head
                        # loads aren't FIFO-blocked behind them on SP
                        eng = nc.scalar if t == NT - 1 else nc.sync
                        eng.dma_start(
                            qkv_t[m][tb][:, tc_ * ST:(tc_ + 1) * ST], ob[:])
                    if t == 3:
                        qt0, kt0 = emit_qk_load(0, 0, eheads)
                        sw0 = emit_swaps(0, 0)
                    elif t == 5:
                        # head 0 ropes here, on the idle DVE, into the early
                        # pool - ready the moment the last qkv matmul retires
                        for op in rope_ops(qt0, kt0, *sw0):
                            op()
                        tiles0[0] = (qt0, kt0,
                                     emit_vkd(0, 0, eheads, tag="vkd0"))

            # ---------------- Phase 2+3: attention with interleaved o_proj ----------------
            with (
                tc.tile_pool(name="headp", bufs=6) as headp,
                tc.tile_pool(name="probsp", bufs=6) as probsp,
                tc.tile_pool(name="accp", bufs=4) as accp,
                tc.tile_pool(name="stagep", bufs=22) as stagep,
                tc.tile_pool(name="miscp", bufs=2) as miscp,
                tc.tile_pool(name="p3w", bufs=1) as wop,
                tc.tile_pool(name="p3o", bufs=4) as p3o,
            ):
                wo_sb = wop.tile([128, NH_LOC, H], BF16, tag="wo")
                wo3 = wo.rearrange("(ko p) f -> p ko f", p=128)

                def emit_wo_load(q):
                    # quarter loads deferred into hooks so head loads aren't
                    # queued behind 4.2MB of weights on the SP FIFO; quarter
                    # q covers o_proj chunks m in [8q, 8q+8)
                    def fn():
                        c0 = q * (H // 4)
                        nc.sync.dma_start(wo_sb[:, :, c0:c0 + H // 4],
                                          wo3[:, :, c0:c0 + H // 4])
                    return fn

                # PE side-work queues. norms (denominator matmuls) must never
                # starve behind o_proj chunks: a group's PSUM/acc slots free
                # only after its normalize runs, and the rings wrap quickly.
                norms = deque()     # entries: (emit_fn, pushed_step)
                fills = deque()
                gstep = [0]         # global pv-step counter across batches

                def make_oproj_chunk(t, m, stages):
                    def emit():
                        ps = ps_acc.tile([128, ST], F32, tag="acc", name=f"ps_o_{t}_{m}")
                        for ko in range(NH_LOC):
                            nc.tensor.matmul(
                                ps[:], wo_sb[:, ko, m * 128:(m + 1) * 128],
                                stages[ko][:],
                                start=(ko == 0), stop=(ko == NH_LOC - 1))
                        ob = p3o.tile([128, ST], BF16, tag="ob3", name=f"ob3_{t}_{m}")
                        nc.scalar.activation(ob[:], ps[:], AF.Copy)
                        nc.sync.dma_start(
                            out[m * 128:(m + 1) * 128, t * ST:(t + 1) * ST], ob[:])
                    return emit

                def make_norm(b, h, g, ps_out, acc, stages):
                    def emit():
                        ps_bc = ps_sc_p.tile([128, ST], F32, tag="sc", name=f"ps_bc_{b}_{h}_{g}")
                        nc.tensor.matmul(ps_bc[:], ones128[:], acc[:],
                                         start=True, stop=True)
                        rec = miscp.tile([128, ST], F32, tag="rec")
                        nc.vector.reciprocal_approx_fast(rec[:], ps_bc[:])
                        stage = stagep.tile([128, ST], BF16, tag="stage",
                                            name=f"stage_{b}_{h}_{g}")
                        nc.vector.tensor_mul(stage[:], ps_out[:], rec[:])
                        stages[g][h] = stage
                    return emit

                def run_batch(b, tiles, pairs, hooks=None):
                    stages = [[None] * NH_LOC for _ in range(GP)]
                    steps = _riffle(pairs)
                    last_of = {}
                    for idx, (h, g, j, nj) in enumerate(steps):
                        if j == nj - 1:
                            last_of[idx] = (h, g)
                    g_seen = [0] * GP
                    grp = {}
                    probs_of = {}

                    def emit_scores(i):
                        h, g, j, nj = steps[i]
                        qt, kt, v_kd = tiles[h]
                        q0 = g * ST
                        r = (j - 4 * g) * 128 if j >= 4 * g else 0
                        w = ST - r
                        if (h, g) not in grp:
                            grp[(h, g)] = (
                                ps_acc.tile([128, ST], F32, tag="acc",
                                            name=f"ps_out_{b}_{h}_{g}"),
                                accp.tile([128, ST], BF16, tag="pacc",
                                          name=f"acc_{b}_{h}_{g}"),
                            )
                        ps_sc = ps_sc_p.tile([128, ST], F32, tag="sc",
                                             name=f"ps_sc_{b}_{h}_{g}_{j}")
                        nc.tensor.matmul(ps_sc[:, r:], kt[:, j * 128:(j + 1) * 128],
                                         qt[:, q0 + r:q0 + ST], start=True, stop=True)
                        probs = probsp.tile([128, ST], BF16, tag="probs",
                                            name=f"probs_{b}_{h}_{g}_{j}")
                        nc.scalar.activation(probs[:, r:], ps_sc[:, r:], AF.Exp,
                                             scale=SCALE)
                        if j >= 4 * g:
                            nc.vector.tensor_mul(
                                probs[:, r:], probs[:, r:], mask_sb[:, 0:w])
                        ps_out, acc = grp[(h, g)]
                        if j == 0:
                            nc.vector.tensor_copy(acc[:], probs[:])
                        else:
                            nc.vector.tensor_add(acc[:, r:], acc[:, r:], probs[:, r:])
                        probs_of[i] = (probs, r)

                    def emit_pv(i):
                        h, g, j, nj = steps[i]
                        qt, kt, v_kd = tiles[h]
                        probs, r = probs_of.pop(i)
                        ps_out, acc = grp[(h, g)]
                        nc.tensor.matmul(ps_out[:, r:], v_kd[:, j], probs[:, r:],
                                         start=(j == 0), stop=(j == nj - 1))
                        if i in last_of:
                            norms.append((make_norm(b, h, g, ps_out, acc, stages),
                                          gstep[0]))
                            g_seen[g] += 1
                            if g_seen[g] == NH_LOC:
                                t = b * GP + g
                                for m in range(H // 128):
                                    fills.append((make_oproj_chunk(t, m, stages[g]),
                                                  gstep[0]))

                    emit_scores(0)
                    emit_scores(1)
                    if b == 0:
                        # warmup filler: keep PE (and its HAM clock) busy for
                        # the ~1.5us the first exp->mask chain needs to fill
                        ps_w = ps_acc.tile([128, ST], F32, tag="acc",
                                           name="ps_warm")
                        for _ in range(5):
                            nc.tensor.matmul(ps_w[:], ones128[:],
                                             cs_sb[:, 0:ST], start=True,
                                             stop=True)
                    for i in range(len(steps)):
                        if hooks and i in hooks:
                            for fn in hooks[i]:
                                fn()
                        if i + 2 < len(steps):
                            emit_scores(i + 2)
                        emit_pv(i)
                        # pop side work: norms queued >=2 steps ago first
                        # (gives the DVE acc chain time), then o_proj chunks.
                        gstep[0] += 1
                        cur = gstep[0]
                        while norms and cur > norms[0][1] + 1:
                            norms.popleft()[0]()
                        npop = 2 if len(fills) > 24 else 1
                        for _ in range(npop):
                            if fills and cur > fills[0][1] + 1:
                                fills.popleft()[0]()

                # b=0: head 0 roped in phase 1; heads 1-3 + all of b=1 chain
                # through the xload ring via step hooks, so their load DMAs
                # and rope ops sit BEHIND the early exps/masks in the ACT and
                # DVE streams instead of blocking them.
                tiles = {0: tiles0[0]}
                tiles1 = {}
                staged = {}

                # q/k load straight into headp qt/kt (no ring pressure); swap
                # tiles rotate through a 2-deep ring (WAR = rope two heads
                # back); rope DVE ops split one-per-hook so masks/accs never
                # queue behind a full 6-op rope.
                hooks0 = {}

                def add_head(b, h, ld_step, rope_steps, vkd_step=None,
                             swap_step=None):
                    sink = tiles if b == 0 else tiles1
                    holder = [None, None, None]

                    def ld():
                        qt, kt = emit_qk_load(b, h, headp)
                        staged[(b, h)] = [qt, kt, None, None]
                        holder[0], holder[1] = qt, kt
                        if swap_step is None:
                            staged[(b, h)][2:] = emit_swaps(b, h)
                        if vkd_step is None:
                            holder[2] = emit_vkd(b, h, headp)
                        sink[h] = holder

                    hooks0.setdefault(ld_step, []).append(ld)
                    if swap_step is not None:
                        def ldsw():
                            # late enough that the ring WAR and data deps are
                            # already met, so the ACT stream never blocks
                            staged[(b, h)][2:] = emit_swaps(b, h, eng=nc.scalar)
                        hooks0.setdefault(swap_step, []).append(ldsw)
                    if vkd_step is not None:
                        def ldv():
                            holder[2] = emit_vkd(b, h, headp)
                        hooks0.setdefault(vkd_step, []).append(ldv)
                    st = {}

                    def mk(k):
                        def fn():
                            if k == 0:
                                st['ops'] = rope_ops(*staged[(b, h)])
                            st['ops'][k]()
                        return fn
                    for k, s in enumerate(rope_steps):
                        hooks0.setdefault(s, []).append(mk(k))

                add_head(0, 1, 1, [3, 5, 7, 9, 11, 13], vkd_step=2)
                add_head(0, 2, 15, [28, 31, 34, 37, 40, 43], vkd_step=36,
                         swap_step=21)
                add_head(0, 3, 17, [46, 48, 50, 52, 54, 56], vkd_step=38,
                         swap_step=25)
                for q in range(4):
                    hooks0.setdefault(44 + 13 * q, []).append(emit_wo_load(q))
                add_head(1, 0, 61, [63, 65, 67, 69, 71, 73])
                add_head(1, 1, 75, [77, 79, 81, 83, 85, 87])
                add_head(1, 2, 89, [91, 93, 95, 97, 99, 101])
                add_head(1, 3, 103, [105, 107, 109, 111, 113, 115])
                run_batch(0, tiles, B0_PAIRS, hooks0)
                run_batch(1, tiles1, B1_PAIRS)

                while norms:
                    norms.popleft()[0]()
                while fills:
                    fills.popleft()[0]()

    nc.finalize()
    return nc


def _prep_inputs(positions, hidden_states, w_pack, w_o):
    pos = np.asarray(positions).astype(np.float32)
    hid = np.asarray(hidden_states, dtype=np.float32)
    w_pack = np.asarray(w_pack, dtype=np.float32)
    w_o = np.asarray(w_o, dtype=np.float32)

    hT = np.ascontiguousarray(hid.reshape(BS, H).T).astype(BF)

    inv_freq = 1.0 / (ROPE_THETA ** (np.arange(0, D, 2, dtype=np.float32) / D))
    ang = pos[None, :] * inv_freq[:, None]              # [64, S]
    cos = np.cos(ang).astype(np.float32)
    sin = np.sin(ang).astype(np.float32)
    cs = np.ascontiguousarray(np.concatenate([cos, cos], 0)).astype(BF)    # [128, S]
    sn = np.ascontiguousarray(np.concatenate([-sin, sin], 0)).astype(BF)

    mask = (np.arange(ST)[None, :] >= np.arange(128)[:, None]).astype(BF)  # [128, 512]

    in_maps = []
    for c in range(NCORES):
        j0 = 512 * c
        w1 = np.concatenate([w_pack[:, j0:j0 + 512],
                             w_pack[:, H + j0:H + j0 + 512],
                             w_pack[:, 2 * H + j0:2 * H + j0 + 512]], axis=1)
        # pack to the SBUF layout [p, m, ko, col]: w1p[p, m, ko, c] = w1[ko*128+p, m*128+c]
        w1p = np.ascontiguousarray(
            w1.reshape(HK, 128, M_QKV, 128).transpose(1, 2, 0, 3)).astype(BF)
        wo = np.ascontiguousarray(w_o[j0:j0 + 512, :]).astype(BF)
        in_maps.append({
            "hT": hT, "w1": w1p, "wo": wo,
            "cs": cs, "sn": sn, "mask": mask,
        })
    return in_maps


def kernel(positions, hidden_states, w_pack, w_o):
    global LAST_RESULT
    nc = _build_program()
    in_maps = _prep_inputs(positions, hidden_states, w_pack, w_o)
    res = run_bass_kernel_spmd(
        nc, in_maps, core_ids=list(range(NCORES)),
        trace=bool(os.environ.get("BASS_TRACE")))
    LAST_RESULT = res
    acc = np.zeros((H, BS), np.float32)
    for r in res.results:
        acc += r["out"].astype(np.float32)
    return np.ascontiguousarray(acc.T).reshape(B, S, H)


# revision 88
# speedup vs baseline: 1.1899x; 1.1899x over previous
"""BaiChuan attention block (QKV proj + RoPE + causal attention + o_proj) on 8 NeuronCores.

Sharding: tensor-parallel over heads. Each core owns 4 of the 32 heads:
W_pack columns (q/k/v slices) are column-sharded, w_o is row-sharded, and the
8 partial o_proj outputs are summed on the host (cheap f32 reduce) instead of
an on-device all-reduce.

Everything on-device runs in bf16 (fp32 PSUM accumulation). Activations are
kept feature-major ("transposed", [feature, batch*seq]) end to end so that
softmax sums run along the PSUM partition axis and no probability-tile
transposes are needed:
  scoresT[k, q] = K_chunk @ Q_group    (lhsT = KT chunk, rhs = QT group)
  probsT = exp(scoresT * scale)        (softmax scale folded into the ACT
                                        scale operand; no max subtraction:
                                        |scores| <= ~12 for this distribution)
  causal mask  = sliding slice of a constant 0/1 tril tile, applied only to
                 the diagonal chunks, which are also column-trimmed: chunk j
                 of group g only computes q columns >= (j-4g)*128
  outT[d, q]  += V_kd chunk @ probsT   (PSUM accumulate over k chunks)
  acc[k, q]   += probsT                (DVE running sum of prob chunks)
  denom+bcast  = ones[128x128] @ acc   (ONE matmul reduces acc over k AND
                                        broadcasts the denominator to all 128
                                        partitions - replaces per-chunk
                                        ones-row matmuls + a bcast matmul)
  normalize    = reciprocal_approx_fast + multiply on DVE

Engine assignment: TensorE matmuls; ScalarE = Exp + PSUM->SBUF copies (exp and
copy live in the same ACT table set, no reload); DVE = rope, masks, prob
accumulation, normalize; sync HWDGE = phase-1 loads (w1 prioritized over ht);
scalar HWDGE = stores + attention loads; gpsimd SWDGE = constants + rope-swap
reads.

Scheduling for the HAM clock gate (engine streams are static and in-order, so
overlap must be baked into emission order):
  - w_pack is host-packed to match SBUF layout (contiguous 8KB rows, no 2x
    small-descriptor DMA penalty), m=0 chunk shipped first so the first
    matmul starts ~4us in.
  - head 0 (and raw head-1 q/k) preload + rope DURING phase 1, into pools
    allocated before the phase-1 pools so addresses never conflict; the
    first scores matmul issues immediately after the last qkv matmul.
  - attention interleaves pairs of (head, group) j-streams ("riffle") plus a
    2-deep scores pipeline, so PE always has >=4 independent matmuls between
    a scores matmul and the PV matmul that needs its exp'd probs.
  - o_proj m-chunks and denominator matmuls are queued side work, popped
    after each PV step (denominators with priority: PSUM/acc rings recycle
    only after a group's normalize).
"""

import os
from collections import deque
import numpy as np
import ml_dtypes

import concourse.bass as bass
import concourse.tile as tile
import concourse.mybir as mybir
from concourse import bacc
from concourse.bass_utils import run_bass_kernel_spmd

F32 = mybir.dt.float32
BF16 = mybir.dt.bfloat16
AF = mybir.ActivationFunctionType
BF = ml_dtypes.bfloat16

B, S, H = 2, 2048, 4096
BS = B * S                      # 4096 tokens
D = 128                         # head dim
NCORES = 8
NH_LOC = 4                      # heads per core (32 / 8)
HK = H // 128                   # 32 contraction chunks for qkv proj
M_QKV = 3 * NH_LOC              # 12 qkv output row-chunks per core
ST = 512                        # seq tile
NT = BS // ST                   # 8 seq tiles
GP = S // ST                    # 4 q-groups per sequence
ROPE_THETA = 10000.0
SCALE = D ** -0.5

# Pairs of (head, group) whose j-streams are riffled together. b=0 opens with
# a head-0-only runway (head 0 is roped during phase 1) while heads 1..3
# load+rope behind it, chained through a single xload buffer ring.
B0_PAIRS = [[(0, 1), (0, 2)], [(0, 0), (0, 3)], [(1, 0), (1, 1)],
            [(2, 0), (1, 2)], [(3, 0), (2, 1)], [(1, 3), (2, 2)],
            [(3, 1), (2, 3)], [(3, 2)], [(3, 3)]]
B1_PAIRS = [[(0, 0), (1, 0)], [(2, 0), (3, 0)], [(0, 1), (1, 1)],
            [(2, 1), (3, 1)], [(0, 2), (1, 2)], [(2, 2), (3, 2)],
            [(0, 3), (1, 3)], [(2, 3), (3, 3)]]

LAST_RESULT = None              # BassKernelResults of the most recent run (for test.py)


def _riffle(pairs):
    """[(h,g)...] pair groups -> flat (h, g, j, nj) step list, interleaved."""
    steps = []
    for grp_list in pairs:
        streams = [[(h, g, j, 4 * g + 4) for j in range(4 * g + 4)]
                   for h, g in grp_list]
        k = 0
        while any(streams):
            st = streams[k % len(streams)]
            if st:
                steps.append(st.pop(0))
            k += 1
    return steps


def _build_program():
    nc = bacc.Bacc()

    hT = nc.dram_tensor("hT", [H, BS], BF16, kind="ExternalInput")
    w1 = nc.dram_tensor("w1", [128, M_QKV, HK, 128], BF16, kind="ExternalInput")
    wo = nc.dram_tensor("wo", [NH_LOC * 128, H], BF16, kind="ExternalInput")
    cs = nc.dram_tensor("cs", [128, S], BF16, kind="ExternalInput")
    sn = nc.dram_tensor("sn", [128, S], BF16, kind="ExternalInput")
    maskd = nc.dram_tensor("mask", [128, ST], BF16, kind="ExternalInput")
    out = nc.dram_tensor("out", [H, BS], BF16, kind="ExternalOutput")

    with tile.TileContext(nc) as tc:
        with (
            tc.tile_pool(name="cons", bufs=1) as cons,
            tc.tile_pool(name="dram", bufs=1, space="DRAM") as dram,
            tc.tile_pool(name="ps_acc", bufs=5, space="PSUM") as ps_acc,
            tc.tile_pool(name="ps_sc", bufs=3, space="PSUM") as ps_sc_p,
            tc.tile_pool(name="xload", bufs=2) as xload,
            tc.tile_pool(name="eheads", bufs=1) as eheads,
        ):
            # per-(row-chunk, batch) bounce tiles: a head's read then depends
            # only on the four writes that filled its own tile, not on the
            # whole phase-1 write stream (interval tracking coarsens columns).
            qkv_t = [[dram.tile([128, S], BF16, name=f"qkv_{m}_{bb}",
                                tag=f"qkv_{m}_{bb}")
                      for bb in range(B)] for m in range(M_QKV)]
            hT3 = hT.rearrange("(ko p) s -> p ko s", p=128)

            def emit_qk_load(b, h, pool):
                """Load raw q/k straight into the rope-target tiles (rope
                then runs in place); all loads on SP so the ScalarE stream
                stays pure exp+copy."""
                nm = f"{b}_{h}"
                qt = pool.tile([128, S], BF16, tag="qt", name=f"qt_{nm}")
                nc.sync.dma_start(qt[:], qkv_t[h][b][:])
                kt = pool.tile([128, S], BF16, tag="kt", name=f"kt_{nm}")
                nc.sync.dma_start(kt[:], qkv_t[NH_LOC + h][b][:])
                return qt, kt

            def emit_swaps(b, h):
                """Partition-swapped q/k copies for the rope sin term."""
                xqs = xload.tile([128, S], BF16, tag="xqs", name=f"xqs_{b}_{h}")
                nc.sync.dma_start(xqs[0:64, :], qkv_t[h][b][64:128, :])
                nc.sync.dma_start(xqs[64:128, :], qkv_t[h][b][0:64, :])
                xks = xload.tile([128, S], BF16, tag="xks", name=f"xks_{b}_{h}")
                nc.sync.dma_start(xks[0:64, :], qkv_t[NH_LOC + h][b][64:128, :])
                nc.sync.dma_start(xks[64:128, :], qkv_t[NH_LOC + h][b][0:64, :])
                return xqs, xks

            def emit_vkd(b, h, pool, tag="vkd"):
                # DmaTranspose holds its issuing sequencer for the whole
                # transfer (~5us): keep these on SP, which has no compute.
                v_kd = pool.tile([128, S // 128, 128], BF16, tag=tag, name=f"vkd_{b}_{h}")
                nc.sync.dma_start_transpose(v_kd[:], qkv_t[2 * NH_LOC + h][b][:])
                return v_kd

            def rope_ops(qt, kt, xqs, xks):
                """Six one-op in-place rope emitters, interleavable with
                attention's mask/acc DVE traffic."""
                return [
                    lambda: nc.vector.tensor_mul(qt[:], qt[:], cs_sb[:]),
                    lambda: nc.vector.tensor_mul(xqs[:], xqs[:], sn_sb[:]),
                    lambda: nc.vector.tensor_add(qt[:], qt[:], xqs[:]),
                    lambda: nc.vector.tensor_mul(kt[:], kt[:], cs_sb[:]),
                    lambda: nc.vector.tensor_mul(xks[:], xks[:], sn_sb[:]),
                    lambda: nc.vector.tensor_add(kt[:], kt[:], xks[:]),
                ]

            # ---------------- Phase 1: qkvT = w1.T @ hT ----------------
            with (
                tc.tile_pool(name="w1p", bufs=1) as w1p,
                tc.tile_pool(name="htp", bufs=2) as htp,
                tc.tile_pool(name="p1o", bufs=2) as p1o,
            ):
                cs_sb = cons.tile([128, S], BF16, tag="cs")
                nc.scalar.dma_start(cs_sb[:], cs[:])
                sn_sb = cons.tile([128, S], BF16, tag="sn")
                nc.scalar.dma_start(sn_sb[:], sn[:])
                mask_sb = cons.tile([128, ST], BF16, tag="mask")
                nc.scalar.dma_start(mask_sb[:], maskd[:])
                ones128 = cons.tile([128, 128], BF16, tag="ones128")
                nc.vector.memset(ones128[:], 1.0)

                w_sb = w1p.tile([128, M_QKV, HK, 128], BF16, tag="w1")

                def emit_ht(t):
                    ht = htp.tile([128, HK, ST], BF16, tag="ht", name=f"ht_{t}")
                    for oct_ in range(4):
                        nc.sync.dma_start(
                            ht[:, oct_ * 8:(oct_ + 1) * 8],
                            hT3[:, oct_ * 8:(oct_ + 1) * 8, t * ST:(t + 1) * ST])
                    return ht

                # first w chunk in ko-quarters riffled with ht0's octs so the
                # very first matmuls are gated by ~1MB, not ~5MB
                nc.sync.dma_start(w_sb[:, 0, 0:8], w1[:, 0, 0:8])
                ht0 = htp.tile([128, HK, ST], BF16, tag="ht", name="ht_0")
                for oct_ in range(4):
                    nc.sync.dma_start(ht0[:, oct_ * 8:(oct_ + 1) * 8],
                                      hT3[:, oct_ * 8:(oct_ + 1) * 8, 0:ST])
                    if oct_ < 3:
                        nc.sync.dma_start(w_sb[:, 0, (oct_ + 1) * 8:(oct_ + 2) * 8],
                                          w1[:, 0, (oct_ + 1) * 8:(oct_ + 2) * 8])
                for m in range(1, M_QKV):
                    nc.sync.dma_start(w_sb[:, m], w1[:, m])

                tiles0 = {}
                ht_next = ht0
                for t in range(NT):
                    ht = ht_next
                    if t < NT - 1:
                        # prefetch emission: ht(t+1) enters the sync FIFO
                        # ahead of tile t's qkv stores
                        ht_next = emit_ht(t + 1)
                    for m in range(M_QKV):
                        ps = ps_acc.tile([128, ST], F32, tag="acc",
                                         name=f"ps_q_{t}_{m}")
                        for ko in range(HK):
                            nc.tensor.matmul(
                                ps[:], w_sb[:, m, ko], ht[:, ko],
                                start=(ko == 0), stop=(ko == HK - 1))
                        ob = p1o.tile([128, ST], BF16, tag="ob")
                        nc.scalar.activation(ob[:], ps[:], AF.Copy)
                        tc_, tb = t % GP, t // GP
                        # t=7 stores issue from the ACT queue (woven between
                        # its own copies, done pre-transition) so the b0 head
                        # loads aren't FIFO-blocked behind them on SP
                        eng = nc.scalar if t == NT - 1 else nc.sync
                        eng.dma_start(
                            qkv_t[m][tb][:, tc_ * ST:(tc_ + 1) * ST], ob[:])
                    if t == 3:
                        qt0, kt0 = emit_qk_load(0, 0, eheads)
                        sw0 = emit_swaps(0, 0)
                    elif t == 5:
                        # head 0 ropes here, on the idle DVE, into the early
                        # pool - ready the moment the last qkv matmul retires
                        for op in rope_ops(qt0, kt0, *sw0):
                            op()
                        tiles0[0] = (qt0, kt0,
                                     emit_vkd(0, 0, eheads, tag="vkd0"))

            # ---------------- Phase 2+3: attention with interleaved o_proj ----------------
            with (
                tc.tile_pool(name="headp", bufs=6) as headp,
                tc.tile_pool(name="probsp", bufs=6) as probsp,
                tc.tile_pool(name="accp", bufs=4) as accp,
                tc.tile_pool(name="stagep", bufs=22) as stagep,
                tc.tile_pool(name="miscp", bufs=2) as miscp,
                tc.tile_pool(name="p3w", bufs=1) as wop,
                tc.tile_pool(name="p3o", bufs=4) as p3o,
            ):
                wo_sb = wop.tile([128, NH_LOC, H], BF16, tag="wo")
                wo3 = wo.rearrange("(ko p) f -> p ko f", p=128)

                def emit_wo_load(q):
                    # quarter loads deferred into hooks so head loads aren't
                    # queued behind 4.2MB of weights on the SP FIFO; quarter
                    # q covers o_proj chunks m in [8q, 8q+8)
                    def fn():
                        c0 = q * (H // 4)
                        nc.sync.dma_start(wo_sb[:, :, c0:c0 + H // 4],
                                          wo3[:, :, c0:c0 + H // 4])
                    return fn

                # PE side-work queues. norms (denominator matmuls) must never
                # starve behind o_proj chunks: a group's PSUM/acc slots free
                # only after its normalize runs, and the rings wrap quickly.
                norms = deque()     # entries: (emit_fn, pushed_step)
                fills = deque()
                gstep = [0]         # global pv-step counter across batches

                def make_oproj_chunk(t, m, stages):
                    def emit():
                        ps = ps_acc.tile([128, ST], F32, tag="acc", name=f"ps_o_{t}_{m}")
                        for ko in range(NH_LOC):
                            nc.tensor.matmul(
                                ps[:], wo_sb[:, ko, m * 128:(m + 1) * 128],
                                stages[ko][:],
                                start=(ko == 0), stop=(ko == NH_LOC - 1))
                        ob = p3o.tile([128, ST], BF16, tag="ob3", name=f"ob3_{t}_{m}")
                        nc.scalar.activation(ob[:], ps[:], AF.Copy)
                        nc.sync.dma_start(
                            out[m * 128:(m + 1) * 128, t * ST:(t + 1) * ST], ob[:])
                    return emit

                def make_norm(b, h, g, ps_out, acc, stages):
                    def emit():
                        ps_bc = ps_sc_p.tile([128, ST], F32, tag="sc", name=f"ps_bc_{b}_{h}_{g}")
                        nc.tensor.matmul(ps_bc[:], ones128[:], acc[:],
                                         start=True, stop=True)
                        rec = miscp.tile([128, ST], F32, tag="rec")
                        nc.vector.reciprocal_approx_fast(rec[:], ps_bc[:])
                        stage = stagep.tile([128, ST], BF16, tag="stage",
                                            name=f"stage_{b}_{h}_{g}")
                        nc.vector.tensor_mul(stage[:], ps_out[:], rec[:])
                        stages[g][h] = stage
                    return emit

                def run_batch(b, tiles, pairs, hooks=None):
                    stages = [[None] * NH_LOC for _ in range(GP)]
                    steps = _riffle(pairs)
                    last_of = {}
                    for idx, (h, g, j, nj) in enumerate(steps):
                        if j == nj - 1:
                            last_of[idx] = (h, g)
                    g_seen = [0] * GP
                    grp = {}
                    probs_of = {}

                    def emit_scores(i):
                        h, g, j, nj = steps[i]
                        qt, kt, v_kd = tiles[h]
                        q0 = g * ST
                        r = (j - 4 * g) * 128 if j >= 4 * g else 0
                        w = ST - r
                        if (h, g) not in grp:
                            grp[(h, g)] = (
                                ps_acc.tile([128, ST], F32, tag="acc",
                                            name=f"ps_out_{b}_{h}_{g}"),
                                accp.tile([128, ST], BF16, tag="pacc",
                                          name=f"acc_{b}_{h}_{g}"),
                            )
                        ps_sc = ps_sc_p.tile([128, ST], F32, tag="sc",
                                             name=f"ps_sc_{b}_{h}_{g}_{j}")
                        nc.tensor.matmul(ps_sc[:, r:], kt[:, j * 128:(j + 1) * 128],
                                         qt[:, q0 + r:q0 + ST], start=True, stop=True)
                        probs = probsp.tile([128, ST], BF16, tag="probs",
                                            name=f"probs_{b}_{h}_{g}_{j}")
                        nc.scalar.activation(probs[:, r:], ps_sc[:, r:], AF.Exp,
                                             scale=SCALE)
                        if j >= 4 * g:
                            nc.vector.tensor_mul(
                                probs[:, r:], probs[:, r:], mask_sb[:, 0:w])
                        ps_out, acc = grp[(h, g)]
                        if j == 0:
                            nc.vector.tensor_copy(acc[:], probs[:])
                        else:
                            nc.vector.tensor_add(acc[:, r:], acc[:, r:], probs[:, r:])
                        probs_of[i] = (probs, r)

                    def emit_pv(i):
                        h, g, j, nj = steps[i]
                        qt, kt, v_kd = tiles[h]
                        probs, r = probs_of.pop(i)
                        ps_out, acc = grp[(h, g)]
                        nc.tensor.matmul(ps_out[:, r:], v_kd[:, j], probs[:, r:],
                                         start=(j == 0), stop=(j == nj - 1))
                        if i in last_of:
                            norms.append((make_norm(b, h, g, ps_out, acc, stages),
                                          gstep[0]))
                            g_seen[g] += 1
                            if g_seen[g] == NH_LOC:
                                t = b * GP + g
                                for m in range(H // 128):
                                    fills.append((make_oproj_chunk(t, m, stages[g]),
                                                  gstep[0]))

                    emit_scores(0)
                    emit_scores(1)
                    if b == 0:
                        # warmup filler: keep PE (and its HAM clock) busy for
                        # the ~1.5us the first exp->mask chain needs to fill
                        ps_w = ps_acc.tile([128, ST], F32, tag="acc",
                                           name="ps_warm")
                        for _ in range(5):
                            nc.tensor.matmul(ps_w[:], ones128[:],
                                             cs_sb[:, 0:ST], start=True,
                                             stop=True)
                    for i in range(len(steps)):
                        if hooks and i in hooks:
                            for fn in hooks[i]:
                                fn()
                        if i + 2 < len(steps):
                            emit_scores(i + 2)
                        emit_pv(i)
                        # pop side work: norms queued >=2 steps ago first
                        # (gives the DVE acc chain time), then o_proj chunks.
                        gstep[0] += 1
                        cur = gstep[0]
                        while norms and cur > norms[0][1] + 1:
                            norms.popleft()[0]()
                        npop = 2 if len(fills) > 24 else 1
                        for _ in range(npop):
                            if fills and cur > fills[0][1] + 1:
                                fills.popleft()[0]()

                # b=0: head 0 roped in phase 1; heads 1-3 + all of b=1 chain
                # through the xload ring via step hooks, so their load DMAs
                # and rope ops sit BEHIND the early exps/masks in the ACT and
                # DVE streams instead of blocking them.
                tiles = {0: tiles0[0]}
                tiles1 = {}
                staged = {}

                # q/k load straight into headp qt/kt (no ring pressure); swap
                # tiles rotate through a 2-deep ring (WAR = rope two heads
                # back); rope DVE ops split one-per-hook so masks/accs never
                # queue behind a full 6-op rope.
                hooks0 = {}

                def add_head(b, h, ld_step, rope_steps, vkd_step=None):
                    sink = tiles if b == 0 else tiles1
                    holder = [None, None, None]

                    def ld():
                        qt, kt = emit_qk_load(b, h, headp)
                        sw = emit_swaps(b, h)
                        staged[(b, h)] = (qt, kt) + sw
                        holder[0], holder[1] = qt, kt
                        if vkd_step is None:
                            holder[2] = emit_vkd(b, h, headp)
                        sink[h] = holder

                    hooks0.setdefault(ld_step, []).append(ld)
                    if vkd_step is not None:
                        def ldv():
                            holder[2] = emit_vkd(b, h, headp)
                        hooks0.setdefault(vkd_step, []).append(ldv)
                    st = {}

                    def mk(k):
                        def fn():
                            if k == 0:
                                st['ops'] = rope_ops(*staged[(b, h)])
                            st['ops'][k]()
                        return fn
                    for k, s in enumerate(rope_steps):
                        hooks0.setdefault(s, []).append(mk(k))

                # SP FIFO order: qt/kt+swap loads for all three heads
                # first (h3's emitted right after rope-h1's ring reads),
                # then the sequencer-holding vkd transposes, then wo
                add_head(0, 1, 1, [3, 5, 7, 9, 11, 13], vkd_step=5)
                add_head(0, 2, 3, [28, 31, 34, 37, 40, 43], vkd_step=6)
                add_head(0, 3, 15, [50, 52, 54, 56, 58, 60], vkd_step=16)
                for q in range(4):
                    hooks0.setdefault(44 + 13 * q, []).append(emit_wo_load(q))
                add_head(1, 0, 61, [63, 65, 67, 69, 71, 73])
                add_head(1, 1, 75, [77, 79, 81, 83, 85, 87])
                add_head(1, 2, 89, [91, 93, 95, 97, 99, 101])
                add_head(1, 3, 103, [105, 107, 109, 111, 113, 115])
                run_batch(0, tiles, B0_PAIRS, hooks0)
                run_batch(1, tiles1, B1_PAIRS)

                while norms:
                    norms.popleft()[0]()
                while fills:
                    fills.popleft()[0]()

    nc.finalize()
    return nc


def _prep_inputs(positions, hidden_states, w_pack, w_o):
    pos = np.asarray(positions).astype(np.float32)
    hid = np.asarray(hidden_states, dtype=np.float32)
    w_pack = np.asarray(w_pack, dtype=np.float32)
    w_o = np.asarray(w_o, dtype=np.float32)

    hT = np.ascontiguousarray(hid.reshape(BS, H).T).astype(BF)

    inv_freq = 1.0 / (ROPE_THETA ** (np.arange(0, D, 2, dtype=np.float32) / D))
    ang = pos[None, :] * inv_freq[:, None]              # [64, S]
    cos = np.cos(ang).astype(np.float32)
    sin = np.sin(ang).astype(np.float32)
    cs = np.ascontiguousarray(np.concatenate([cos, cos], 0)).astype(BF)    # [128, S]
    sn = np.ascontiguousarray(np.concatenate([-sin, sin], 0)).astype(BF)

    mask = (np.arange(ST)[None, :] >= np.arange(128)[:, None]).astype(BF)  # [128, 512]

    in_maps = []
    for c in range(NCORES):
        j0 = 512 * c
        w1 = np.concatenate([w_pack[:, j0:j0 + 512],
                             w_pack[:, H + j0:H + j0 + 512],
                             w_pack[:, 2 * H + j0:2 * H + j0 + 512]], axis=1)
        # pack to the SBUF layout [p, m, ko, col]: w1p[p, m, ko, c] = w1[ko*128+p, m*128+c]
        w1p = np.ascontiguousarray(
            w1.reshape(HK, 128, M_QKV, 128).transpose(1, 2, 0, 3)).astype(BF)
        wo = np.ascontiguousarray(w_o[j0:j0 + 512, :]).astype(BF)
        in_maps.append({
            "hT": hT, "w1": w1p, "wo": wo,
            "cs": cs, "sn": sn, "mask": mask,
        })
    return in_maps


def kernel(positions, hidden_states, w_pack, w_o):
    global LAST_RESULT
    nc = _build_program()
    in_maps = _prep_inputs(positions, hidden_states, w_pack, w_o)
    res = run_bass_kernel_spmd(
        nc, in_maps, core_ids=list(range(NCORES)),
        trace=bool(os.environ.get("BASS_TRACE")))
    LAST_RESULT = res
    acc = np.zeros((H, BS), np.float32)
    for r in res.results:
        acc += r["out"].astype(np.float32)
    return np.ascontiguousarray(acc.T).reshape(B, S, H)
